# revision 18
# baseline (speedup 1.0000x reference)
"""Supervised-contrastive loss on 8 TRN2 NeuronCores — v2.

Math (identical to the reference up to a validated ~2e-5 approximation):
    s_ij  = cosine similarity, E_ij = exp(s_ij/tau)
    neg_i = sum_j E_ij * (1 - mask_ij)
    loss  = sum_{i, j in pos(i), j != i} [ln(E_ij + neg_i) - s_ij/tau] / p_i
            ----------------------------------------------------------------
                                     sum_i p_i
Since E_ij / neg_i ~ 1e-3 for off-diagonal positives,
    ln(E_ij + neg_i) = ln(neg_i) + E_ij/neg_i + O((E/neg)^2),
so per row only THREE scalars are needed from the device:
    rsE_i  = sum_j E_ij           (free accumulator on the exp pass)
    rsEM_i = sum_{j in pos} E_ij  (masked window sum, incl. diagonal)
    Eii_i  = E_ii                 (diagonal pick)
and the host finishes in f64:
    neg = rsE - rsEM;  S2 = rsEM - Eii
    numer_i = (p_i-1) ln(neg_i) + S2_i/neg_i - (fn_i.g(t_i) - 1)/tau
    loss = sum(numer/p) / sum(p)

Device layout tricks:
  * Rows AND columns sorted by class (host) -> the positive mask is block
    diagonal. Each core owns 512 sorted rows; its input column order is
    ROTATED by (c*512 - 128) so every core sees its own class block at a
    fixed window: all positives of its rows live in rotated columns
    [0, 768) and the diagonal of row-tile `it` is at column 128+it*128+p.
    The expensive DVE mask pass thus shrinks from 4096 to 768 columns.
  * GEMM in fp8e4 (inputs scaled x16) with DoubleRow perf mode:
    contraction 512 = 2 supertiles of 256 (2 fp8 weights per PE cell).
  * exp on ACT with fused row-accumulate gives rsE for free.
No second activation pass, no ln tables, no full-size mask multiply.
"""

import numpy as np
import ml_dtypes

TAU = 0.1
N, D = 4096, 512
NCORES = 8
ROWS = N // NCORES          # 512 rows per core
ITILES = ROWS // 128        # 4 partition tiles per core
CC = 2                      # two 2048-wide column chunks
KK = 2                      # two 256-deep contraction supertiles
SCALE = 16.0                # fp8 pre-scale; exp() scale compensates
WIN = 768                   # masked-window width (needs max class size <= 128)

_CACHE = {}


def _build_nc():
    import concourse.tile as tile
    import concourse.mybir as mybir
    from concourse import bacc

    dt = mybir.dt
    AF = mybir.ActivationFunctionType
    ALU = mybir.AluOpType
    AX = mybir.AxisListType
    PM = mybir.MatmulPerfMode

    nc = bacc.Bacc(None)
    # fnr: rotated+scaled fp8 features, packed chunk-contiguously so each
    # (cc, kk) group is ONE dma:  fnr[p, ((cc*2+kk)*2+pl)*2048 + j] =
    #   fnT_rot[kk*256 + pl*128 + p, cc*2048 + j]
    fnr = nc.declare_dram_parameter("fnr", [128, 8, 2048], dt.float8e4, isOutput=False)
    # lhs: packed per kk: lhs[p, (kk*2+pl)*ROWS + m]
    lhs = nc.declare_dram_parameter("lhs", [128, 4 * ROWS], dt.float8e4, isOutput=False)
    tb = nc.declare_dram_parameter("tb", [128, WIN], dt.bfloat16, isOutput=False)
    tcol = nc.declare_dram_parameter("tcol", [128, ITILES], dt.float32, isOutput=False)
    iot = nc.declare_dram_parameter("iot", [128, 128], dt.bfloat16, isOutput=False)
    prow = nc.declare_dram_parameter("prow", [128, 1], dt.float32, isOutput=False)
    # merged output: [rse | rsem | eii] each ITILES wide
    res_out = nc.declare_dram_parameter("res_out", [128, 3 * ITILES], dt.float32, isOutput=True)

    with tile.TileContext(nc) as tc:
        with (
            tc.tile_pool(name="persist", bufs=1) as persist,
            tc.tile_pool(name="psum", bufs=2, space="PSUM") as psum,
            tc.tile_pool(name="ebuf", bufs=3) as ebuf,
            tc.tile_pool(name="scr", bufs=2) as scr,
            tc.tile_pool(name="acc", bufs=1) as accp,
            tc.tile_pool(name="outp", bufs=1) as outp,
        ):
            # ---- persistent loads; GEMM-blocking first, split for overlap ----
            # One DMA per (cc, kk) fn group; kk0 on sync, kk1 on gpsimd so
            # the first accumulation group's data lands first.
            fn_sb = [[None] * KK for _ in range(CC)]
            lhs_sb = [None] * KK
            with tc.high_priority():
                for kk in range(KK):
                    lhs_sb[kk] = persist.tile([128, 2, ROWS], dt.float8e4,
                                              name=f"lhs_{kk}", tag=f"lhs_{kk}")
                for cc in range(CC):
                    for kk in range(KK):
                        fn_sb[cc][kk] = persist.tile(
                            [128, 2, 2048], dt.float8e4,
                            name=f"fn_{cc}_{kk}", tag=f"fn_{cc}_{kk}")
                eng = {0: nc.sync, 1: nc.gpsimd}
                for kk in range(KK):
                    eng[kk].dma_start(
                        lhs_sb[kk][:, :, :],
                        lhs[:, kk * 2 * ROWS:(kk + 1) * 2 * ROWS])
                # chunk 0: 512-col pieces matching the matmul slices so the
                # first matmul starts after one small piece lands
                for nb in range(4):
                    for kk in range(KK):
                        eng[kk].dma_start(
                            fn_sb[0][kk][:, :, nb * 512:(nb + 1) * 512],
                            fnr[:, kk * 2:(kk + 1) * 2, nb * 512:(nb + 1) * 512])
                # chunk 1 + small tensors on the otherwise-idle scalar queue
                nc.scalar.dma_start(fn_sb[1][0][:, :, :], fnr[:, 4:6, :])
                nc.scalar.dma_start(fn_sb[1][1][:, :, :], fnr[:, 6:8, :])
                tcol_sb = persist.tile([128, ITILES], dt.float32, tag="tcol")
                nc.scalar.dma_start(tcol_sb[:], tcol[:])
                tb_sb = persist.tile([128, WIN], dt.bfloat16, tag="tb")
                nc.scalar.dma_start(tb_sb[:], tb[:])
                iot_sb = persist.tile([128, 128], dt.bfloat16, tag="iot")
                nc.scalar.dma_start(iot_sb[:], iot[:])
                prow_sb = persist.tile([128, 1], dt.float32, tag="prow")
                nc.scalar.dma_start(prow_sb[:], prow[:])

            res_sb = outp.tile([128, 3 * ITILES], dt.float32, tag="res")
            rse2 = [accp.tile([128, CC], dt.float32, name=f"rse2_{it}",
                              tag=f"rse2_{it}")
                    for it in range(ITILES)]

            # ---- GEMM + exp(+rsE accum) + windowed mask sums ----
            # cc-outer so all chunk-0 work runs first (its fn data lands first)
            for cc in range(CC):
                for it in range(ITILES):
                    S = psum.tile([128, 2048], dt.float32, tag="S")
                    for kk in range(KK):
                        for nb in range(4):
                            nc.tensor.matmul(
                                S[:, nb * 512:(nb + 1) * 512],
                                lhs_sb[kk][:, :, it * 128:(it + 1) * 128],
                                fn_sb[cc][kk][:, :, nb * 512:(nb + 1) * 512],
                                start=(kk == 0),
                                stop=(kk == KK - 1),
                                perf_mode=PM.DoubleRow,
                            )
                    E = ebuf.tile([128, 2048], dt.bfloat16, tag="E")
                    nc.scalar.activation(
                        E[:], S[:], AF.Exp, scale=1.0 / (TAU * SCALE * SCALE),
                        accum_out=rse2[it][:, cc:cc + 1],
                    )
                    if cc == 0:
                        em_scr = scr.tile([128, WIN], dt.bfloat16, tag="em_scr")
                        nc.vector.scalar_tensor_tensor(
                            em_scr[:], tb_sb[:], tcol_sb[:, it:it + 1], E[:, 0:WIN],
                            ALU.is_equal, ALU.mult,
                            accum_out=res_sb[:, ITILES + it:ITILES + it + 1],
                        )
                        d_scr = scr.tile([128, 128], dt.bfloat16, tag="d_scr")
                        nc.vector.scalar_tensor_tensor(
                            d_scr[:], iot_sb[:], prow_sb[:],
                            E[:, 128 + it * 128:256 + it * 128],
                            ALU.is_equal, ALU.mult,
                            accum_out=res_sb[:, 2 * ITILES + it:2 * ITILES + it + 1],
                        )
                    else:
                        nc.vector.tensor_reduce(
                            res_sb[:, it:it + 1], rse2[it][:], AX.X, ALU.add
                        )

            nc.sync.dma_start(res_out[:], res_sb[:])

    nc.finalize()
    return nc


def _get_nc():
    if "nc" not in _CACHE:
        _CACHE["nc"] = _build_nc()
    return _CACHE["nc"]


def _host_prep(features, targets):
    f8 = ml_dtypes.float8_e4m3
    bf16 = ml_dtypes.bfloat16
    f = np.asarray(features, np.float32)
    t = np.asarray(targets).astype(np.int64)
    idx = np.argsort(t, kind="stable")
    ts = t[idx]
    assert np.bincount(ts).max() <= 128, "class size exceeds mask window"
    rnorm = 1.0 / np.sqrt((f.astype(np.float64) ** 2).sum(1))
    fn = (f * rnorm[:, None].astype(np.float32)).astype(np.float32)
    fns = fn[idx]
    q = (fns * SCALE).astype(f8)                     # [N, D] fp8 scaled
    qT = np.ascontiguousarray(q.T)                   # [D, N]
    tsb = ts.astype(np.float32)

    iot = np.broadcast_to(np.arange(128, dtype=np.float32).astype(bf16)[None, :],
                          (128, 128))
    prow = np.arange(128, dtype=np.float32).reshape(128, 1)

    def pack_lhs(m):                                 # [D, C] -> [128, 4*C]
        cdim = m.shape[1]
        out = np.empty((128, 4 * cdim), f8)
        for kk in range(KK):
            for pl in range(2):
                r0 = kk * 256 + pl * 128
                out[:, (kk * 2 + pl) * cdim:(kk * 2 + pl + 1) * cdim] = \
                    m[r0:r0 + 128, :]
        return out

    def pack_fn(m):                                  # [D, N] -> [128, 8, 2048]
        out = np.empty((128, 8, 2048), f8)
        for cc in range(CC):
            for kk in range(KK):
                for pl in range(2):
                    r0 = kk * 256 + pl * 128
                    out[:, (cc * 2 + kk) * 2 + pl, :] = \
                        m[r0:r0 + 128, cc * 2048:(cc + 1) * 2048]
        return out

    in_maps = []
    for c in range(NCORES):
        rot = (np.arange(N) + c * ROWS - 128) % N
        in_maps.append({
            "fnr": pack_fn(qT[:, rot]),
            "lhs": pack_lhs(qT[:, c * ROWS:(c + 1) * ROWS]),
            "tb": np.ascontiguousarray(
                np.broadcast_to(tsb[rot[:WIN]].astype(bf16)[None, :], (128, WIN))),
            "tcol": np.ascontiguousarray(
                tsb[c * ROWS:(c + 1) * ROWS].reshape(ITILES, 128).T),
            "iot": np.ascontiguousarray(iot),
            "prow": prow,
        })
    return fns, ts, in_maps


def _host_post(fns, ts, rse, rsem, eii):
    # rse/rsem/eii: [N] float64 in sorted-row order
    p = np.bincount(ts)[ts].astype(np.float64)
    neg = rse - rsem
    s2 = rsem - eii
    g = np.zeros((int(ts.max()) + 1, D), np.float64)
    np.add.at(g, ts, fns.astype(np.float64))
    dotg = (fns.astype(np.float64) * g[ts]).sum(1)
    numer = (p - 1.0) * np.log(neg) + s2 / neg - (dotg - 1.0) / TAU
    loss = (numer / p).sum() / p.sum()
    return np.float32(loss)


def _rows_from_out(per_core_outs, sect):
    # [128, 3*ITILES] per core; section sect in {0:rse, 1:rsem, 2:eii};
    # row index = core*512 + it*128 + p
    rows = np.empty(N, np.float64)
    for c, out in enumerate(per_core_outs):
        arr = np.asarray(out["res_out"], np.float64)[:, sect * ITILES:(sect + 1) * ITILES]
        rows[c * ROWS:(c + 1) * ROWS] = arr.T.reshape(ROWS)
    return rows


def _run(in_maps, trace=False):
    from concourse.bass_utils import run_bass_kernel_spmd
    nc = _get_nc()
    return run_bass_kernel_spmd(
        nc, in_maps, core_ids=list(range(NCORES)), trace=trace,
    )


def kernel(features, targets):
    fns, ts, in_maps = _host_prep(features, targets)
    res = _run(in_maps, trace=False)
    rse = _rows_from_out(res.results, 0)
    rsem = _rows_from_out(res.results, 1)
    eii = _rows_from_out(res.results, 2)
    return _host_post(fns, ts, rse, rsem, eii)


# revision 19
# speedup vs baseline: 1.1034x; 1.1034x over previous
"""Supervised-contrastive loss on 8 TRN2 NeuronCores — v2.

Math (identical to the reference up to a validated ~2e-5 approximation):
    s_ij  = cosine similarity, E_ij = exp(s_ij/tau)
    neg_i = sum_j E_ij * (1 - mask_ij)
    loss  = sum_{i, j in pos(i), j != i} [ln(E_ij + neg_i) - s_ij/tau] / p_i
            ----------------------------------------------------------------
                                     sum_i p_i
Since E_ij / neg_i ~ 1e-3 for off-diagonal positives,
    ln(E_ij + neg_i) = ln(neg_i) + E_ij/neg_i + O((E/neg)^2),
so per row only THREE scalars are needed from the device:
    rsE_i  = sum_j E_ij           (free accumulator on the exp pass)
    rsEM_i = sum_{j in pos} E_ij  (masked window sum, incl. diagonal)
    Eii_i  = E_ii                 (diagonal pick)
and the host finishes in f64:
    neg = rsE - rsEM;  S2 = rsEM - Eii
    numer_i = (p_i-1) ln(neg_i) + S2_i/neg_i - (fn_i.g(t_i) - 1)/tau
    loss = sum(numer/p) / sum(p)

Device layout tricks:
  * Rows AND columns sorted by class (host) -> the positive mask is block
    diagonal. Each core owns 512 sorted rows; its input column order is
    ROTATED by (c*512 - 128) so every core sees its own class block at a
    fixed window: all positives of its rows live in rotated columns
    [0, 768) and the diagonal of row-tile `it` is at column 128+it*128+p.
    The expensive DVE mask pass thus shrinks from 4096 to 768 columns.
  * GEMM in fp8e4 (inputs scaled x16) with DoubleRow perf mode:
    contraction 512 = 2 supertiles of 256 (2 fp8 weights per PE cell).
  * exp on ACT with fused row-accumulate gives rsE for free.
No second activation pass, no ln tables, no full-size mask multiply.
"""

import numpy as np
import ml_dtypes

TAU = 0.1
N, D = 4096, 512
NCORES = 8
ROWS = N // NCORES          # 512 rows per core
ITILES = ROWS // 128        # 4 partition tiles per core
CC = 2                      # two 2048-wide column chunks
KK = 2                      # two 256-deep contraction supertiles
SCALE = 16.0                # fp8 pre-scale; exp() scale compensates
WIN = 768                   # masked-window width (needs max class size <= 128)

_CACHE = {}


def _build_nc():
    import concourse.tile as tile
    import concourse.mybir as mybir
    from concourse import bacc

    dt = mybir.dt
    AF = mybir.ActivationFunctionType
    ALU = mybir.AluOpType
    AX = mybir.AxisListType
    PM = mybir.MatmulPerfMode

    nc = bacc.Bacc(None)
    # fnr: rotated+scaled fp8 features, packed chunk-contiguously so each
    # (cc, kk) group is ONE dma:  fnr[p, ((cc*2+kk)*2+pl)*2048 + j] =
    #   fnT_rot[kk*256 + pl*128 + p, cc*2048 + j]
    fnr = nc.declare_dram_parameter("fnr", [128, 8, 2048], dt.float8e4, isOutput=False)
    # lhs: packed per kk: lhs[p, (kk*2+pl)*ROWS + m]
    lhs = nc.declare_dram_parameter("lhs", [128, 4 * ROWS], dt.float8e4, isOutput=False)
    tb = nc.declare_dram_parameter("tb", [128, WIN], dt.bfloat16, isOutput=False)
    tcol = nc.declare_dram_parameter("tcol", [128, ITILES], dt.float32, isOutput=False)
    iot = nc.declare_dram_parameter("iot", [128, 128], dt.bfloat16, isOutput=False)
    prow = nc.declare_dram_parameter("prow", [128, 1], dt.float32, isOutput=False)
    # merged output: [rse | rsem | eii] each ITILES wide
    res_out = nc.declare_dram_parameter("res_out", [128, 3 * ITILES], dt.float32, isOutput=True)

    with tile.TileContext(nc) as tc:
        with (
            tc.tile_pool(name="persist", bufs=1) as persist,
            tc.tile_pool(name="psum", bufs=2, space="PSUM") as psum,
            tc.tile_pool(name="ebuf", bufs=3) as ebuf,
            tc.tile_pool(name="scr", bufs=2) as scr,
            tc.tile_pool(name="acc", bufs=1) as accp,
            tc.tile_pool(name="outp", bufs=1) as outp,
        ):
            # ---- persistent loads; GEMM-blocking first, split for overlap ----
            # One DMA per (cc, kk) fn group; kk0 on sync, kk1 on gpsimd so
            # the first accumulation group's data lands first.
            fn_sb = [[None] * KK for _ in range(CC)]
            lhs_sb = [None] * KK
            with tc.high_priority():
                for kk in range(KK):
                    lhs_sb[kk] = persist.tile([128, 2, ROWS], dt.float8e4,
                                              name=f"lhs_{kk}", tag=f"lhs_{kk}")
                for cc in range(CC):
                    for kk in range(KK):
                        fn_sb[cc][kk] = persist.tile(
                            [128, 2, 2048], dt.float8e4,
                            name=f"fn_{cc}_{kk}", tag=f"fn_{cc}_{kk}")
                eng = {0: nc.sync, 1: nc.gpsimd}
                for kk in range(KK):
                    eng[kk].dma_start(
                        lhs_sb[kk][:, :, :],
                        lhs[:, kk * 2 * ROWS:(kk + 1) * 2 * ROWS])
                # chunk 0: 512-col pieces matching the matmul slices so the
                # first matmul starts after one small piece lands
                for nb in range(4):
                    for kk in range(KK):
                        eng[kk].dma_start(
                            fn_sb[0][kk][:, :, nb * 512:(nb + 1) * 512],
                            fnr[:, kk * 2:(kk + 1) * 2, nb * 512:(nb + 1) * 512])
                # chunk 1 back on sync/gpsimd; small tensors on gpsimd
                nc.sync.dma_start(fn_sb[1][0][:, :, :], fnr[:, 4:6, :])
                tcol_sb = persist.tile([128, ITILES], dt.float32, tag="tcol")
                nc.gpsimd.dma_start(tcol_sb[:], tcol[:])
                tb_sb = persist.tile([128, WIN], dt.bfloat16, tag="tb")
                nc.gpsimd.dma_start(tb_sb[:], tb[:])
                iot_sb = persist.tile([128, 128], dt.bfloat16, tag="iot")
                nc.gpsimd.dma_start(iot_sb[:], iot[:])
                prow_sb = persist.tile([128, 1], dt.float32, tag="prow")
                nc.gpsimd.dma_start(prow_sb[:], prow[:])
                nc.gpsimd.dma_start(fn_sb[1][1][:, :, :], fnr[:, 6:8, :])

            res_sb = outp.tile([128, 3 * ITILES], dt.float32, tag="res")
            rse2 = [accp.tile([128, CC], dt.float32, name=f"rse2_{it}",
                              tag=f"rse2_{it}")
                    for it in range(ITILES)]

            # ---- GEMM + exp(+rsE accum) + windowed mask sums ----
            # cc-outer so all chunk-0 work runs first (its fn data lands first)
            for cc in range(CC):
                for it in range(ITILES):
                    S = psum.tile([128, 2048], dt.float32, tag="S")
                    for kk in range(KK):
                        for nb in range(4):
                            nc.tensor.matmul(
                                S[:, nb * 512:(nb + 1) * 512],
                                lhs_sb[kk][:, :, it * 128:(it + 1) * 128],
                                fn_sb[cc][kk][:, :, nb * 512:(nb + 1) * 512],
                                start=(kk == 0),
                                stop=(kk == KK - 1),
                                perf_mode=PM.DoubleRow,
                            )
                    E = ebuf.tile([128, 2048], dt.bfloat16, tag="E")
                    nc.scalar.activation(
                        E[:], S[:], AF.Exp, scale=1.0 / (TAU * SCALE * SCALE),
                        accum_out=rse2[it][:, cc:cc + 1],
                    )
                    if cc == 0:
                        em_scr = scr.tile([128, WIN], dt.bfloat16, tag="em_scr")
                        nc.vector.scalar_tensor_tensor(
                            em_scr[:], tb_sb[:], tcol_sb[:, it:it + 1], E[:, 0:WIN],
                            ALU.is_equal, ALU.mult,
                            accum_out=res_sb[:, ITILES + it:ITILES + it + 1],
                        )
                        d_scr = scr.tile([128, 128], dt.bfloat16, tag="d_scr")
                        nc.vector.scalar_tensor_tensor(
                            d_scr[:], iot_sb[:], prow_sb[:],
                            E[:, 128 + it * 128:256 + it * 128],
                            ALU.is_equal, ALU.mult,
                            accum_out=res_sb[:, 2 * ITILES + it:2 * ITILES + it + 1],
                        )
                    else:
                        nc.vector.tensor_reduce(
                            res_sb[:, it:it + 1], rse2[it][:], AX.X, ALU.add
                        )

            nc.sync.dma_start(res_out[:], res_sb[:])

    nc.finalize()
    return nc


def _get_nc():
    if "nc" not in _CACHE:
        _CACHE["nc"] = _build_nc()
    return _CACHE["nc"]


def _host_prep(features, targets):
    f8 = ml_dtypes.float8_e4m3
    bf16 = ml_dtypes.bfloat16
    f = np.asarray(features, np.float32)
    t = np.asarray(targets).astype(np.int64)
    idx = np.argsort(t, kind="stable")
    ts = t[idx]
    assert np.bincount(ts).max() <= 128, "class size exceeds mask window"
    rnorm = 1.0 / np.sqrt((f.astype(np.float64) ** 2).sum(1))
    fn = (f * rnorm[:, None].astype(np.float32)).astype(np.float32)
    fns = fn[idx]
    q = (fns * SCALE).astype(f8)                     # [N, D] fp8 scaled
    qT = np.ascontiguousarray(q.T)                   # [D, N]
    tsb = ts.astype(np.float32)

    iot = np.broadcast_to(np.arange(128, dtype=np.float32).astype(bf16)[None, :],
                          (128, 128))
    prow = np.arange(128, dtype=np.float32).reshape(128, 1)

    def pack_lhs(m):                                 # [D, C] -> [128, 4*C]
        cdim = m.shape[1]
        out = np.empty((128, 4 * cdim), f8)
        for kk in range(KK):
            for pl in range(2):
                r0 = kk * 256 + pl * 128
                out[:, (kk * 2 + pl) * cdim:(kk * 2 + pl + 1) * cdim] = \
                    m[r0:r0 + 128, :]
        return out

    def pack_fn(m):                                  # [D, N] -> [128, 8, 2048]
        out = np.empty((128, 8, 2048), f8)
        for cc in range(CC):
            for kk in range(KK):
                for pl in range(2):
                    r0 = kk * 256 + pl * 128
                    out[:, (cc * 2 + kk) * 2 + pl, :] = \
                        m[r0:r0 + 128, cc * 2048:(cc + 1) * 2048]
        return out

    in_maps = []
    for c in range(NCORES):
        rot = (np.arange(N) + c * ROWS - 128) % N
        in_maps.append({
            "fnr": pack_fn(qT[:, rot]),
            "lhs": pack_lhs(qT[:, c * ROWS:(c + 1) * ROWS]),
            "tb": np.ascontiguousarray(
                np.broadcast_to(tsb[rot[:WIN]].astype(bf16)[None, :], (128, WIN))),
            "tcol": np.ascontiguousarray(
                tsb[c * ROWS:(c + 1) * ROWS].reshape(ITILES, 128).T),
            "iot": np.ascontiguousarray(iot),
            "prow": prow,
        })
    return fns, ts, in_maps


def _host_post(fns, ts, rse, rsem, eii):
    # rse/rsem/eii: [N] float64 in sorted-row order
    p = np.bincount(ts)[ts].astype(np.float64)
    neg = rse - rsem
    s2 = rsem - eii
    g = np.zeros((int(ts.max()) + 1, D), np.float64)
    np.add.at(g, ts, fns.astype(np.float64))
    dotg = (fns.astype(np.float64) * g[ts]).sum(1)
    numer = (p - 1.0) * np.log(neg) + s2 / neg - (dotg - 1.0) / TAU
    loss = (numer / p).sum() / p.sum()
    return np.float32(loss)


def _rows_from_out(per_core_outs, sect):
    # [128, 3*ITILES] per core; section sect in {0:rse, 1:rsem, 2:eii};
    # row index = core*512 + it*128 + p
    rows = np.empty(N, np.float64)
    for c, out in enumerate(per_core_outs):
        arr = np.asarray(out["res_out"], np.float64)[:, sect * ITILES:(sect + 1) * ITILES]
        rows[c * ROWS:(c + 1) * ROWS] = arr.T.reshape(ROWS)
    return rows


def _run(in_maps, trace=False):
    from concourse.bass_utils import run_bass_kernel_spmd
    nc = _get_nc()
    return run_bass_kernel_spmd(
        nc, in_maps, core_ids=list(range(NCORES)), trace=trace,
    )


def kernel(features, targets):
    fns, ts, in_maps = _host_prep(features, targets)
    res = _run(in_maps, trace=False)
    rse = _rows_from_out(res.results, 0)
    rsem = _rows_from_out(res.results, 1)
    eii = _rows_from_out(res.results, 2)
    return _host_post(fns, ts, rse, rsem, eii)
